# revision 26
# baseline (speedup 1.0000x reference)
"""GCN layer (symmetric-normalized, self-loops) on 8 Trainium2 NeuronCores.

out[d] = sum_{e:(s,d)} rsqrt(deg_s*deg_d) * (h_s @ W.T + b)

Factorization (linearity of the edge aggregation), rs = deg**-0.5:
  out[d] = rs_d * [ (sum_e rs_s h_s) @ W.T + (sum_e rs_s) * b ]

Device strategy (dst-sharded, SPMD over 8 cores, one instruction stream):
  - h is pre-scaled by rs_src per row (h' = diag(rs) h, folded into the bf16
    cast) so the edge one-hot S is BINARY and can be fp8 (1 byte, exact 1.0).
  - nodes are bin-packed into fixed windows of <=128 dst nodes.
  - per chunk of 128 edges: dma_gather 128 rows of h' into SBUF partitions
    (indices are signed int16, so h is split into h0=h[:32768] / h1=rest;
    2048-idx gathers amortize the SWDGE fixed overhead; 4 gathers share one
    packed index DMA), and accumulate P^T += X'^T @ S on the PE
    (lhsT = gathered X' chunk bf16, rhs = S one-hot fp8 from DRAM).
  - per window: P^T [feat, 128] PSUM -> SBUF (ACT); out_ps = P^T.T @ W.T on
    the PE; tmp = psum + wsum'_d*b on the DVE (rank-1 bias, reads PSUM);
    out = rs_d * tmp on the ACT copy (per-partition scale), DMA out as bf16.
Host (numpy) does only index/graph-metadata preparation + the diagonal
rs pre-scale folded into the input cast.
"""

import sys

sys.path.insert(0, "/opt/trn_rl_repo")

import heapq

import numpy as np

N_NODES = 50000
D = 128
N_CORES = 8
H0 = 32768          # rows in first gather table (int16 index limit)
P = 128
KCH = 8             # chunks per dma_gather (1024 indices = SWDGE max)
GPK = 4             # gathers per packed index DMA

_COMPILED = {}


def _pack_windows(c0, c1, n_bins, cap0, cap1):
    """Assign each node to a window (bin) s.t. per-bin sums of c0/c1 stay
    under cap0/cap1 and <=128 nodes per bin.  Worst-fit greedy on the max
    utilization of the two capacities, largest nodes first."""
    order = np.argsort(-(c0 + c1), kind="stable")
    heap = [(0.0, b) for b in range(n_bins)]
    heapq.heapify(heap)
    used0 = np.zeros(n_bins, np.int64)
    used1 = np.zeros(n_bins, np.int64)
    ncnt = np.zeros(n_bins, np.int64)
    win_of = np.full(N_NODES, -1, np.int32)
    for node in order:
        a = int(c0[node])
        b = int(c1[node])
        popped = []
        placed = False
        while heap:
            _, bidx = heapq.heappop(heap)
            if used0[bidx] + a <= cap0 and used1[bidx] + b <= cap1 and ncnt[bidx] < P:
                used0[bidx] += a
                used1[bidx] += b
                ncnt[bidx] += 1
                win_of[node] = bidx
                key = max(used0[bidx] / cap0, used1[bidx] / cap1)
                heapq.heappush(heap, (key, bidx))
                placed = True
                break
            if ncnt[bidx] < P:
                popped.append((max(used0[bidx] / cap0, used1[bidx] / cap1), bidx))
            # bins at node capacity are dropped permanently
        for it in popped:
            heapq.heappush(heap, it)
        if not placed:
            return None
    return win_of


def _wrap_idx(idx_flat):
    """dma_gather index layout: idx i -> partition i%16, col i//16,
    replicated 8x across the 128 partitions (one copy per gpsimd core)."""
    w = idx_flat.reshape(-1, 16).T
    return np.ascontiguousarray(np.tile(w, (8, 1)))


def _preprocess(h, W, b, edges):
    import ml_dtypes

    src = np.concatenate([edges[0], np.arange(N_NODES, dtype=np.int64)]).astype(np.int64)
    dst = np.concatenate([edges[1], np.arange(N_NODES, dtype=np.int64)]).astype(np.int64)
    n_e = src.shape[0]

    deg = np.bincount(dst, minlength=N_NODES).astype(np.float32)
    rs = (deg ** -0.5).astype(np.float32)
    # wsum'[d] = sum_{e into d} rs_src  (bias coefficient; rs_d on ACT copy)
    wsum_full = np.bincount(dst, weights=rs[src].astype(np.float64),
                            minlength=N_NODES).astype(np.float32)

    half = (src >= H0).astype(np.int64)
    c0 = np.bincount(dst[half == 0], minlength=N_NODES)
    c1 = np.bincount(dst[half == 1], minlength=N_NODES)

    n_win = 52                              # windows per core
    n_win_tot = n_win * N_CORES
    for T0, T1 in ((11, 6), (11, 7), (12, 7), (13, 8)):
        win_of = _pack_windows(c0, c1, n_win_tot, T0 * P, T1 * P)
        if win_of is not None:
            break
    assert win_of is not None, "window packing failed"

    slot_of = np.zeros(N_NODES, np.int32)
    win_nodes_count = np.zeros(n_win_tot, np.int32)
    order = np.argsort(win_of, kind="stable")
    for node in order:
        wg = win_of[node]
        slot_of[node] = win_nodes_count[wg]
        win_nodes_count[wg] += 1

    NC0, NC1 = n_win * T0, n_win * T1       # chunks per core per stream
    NG0 = -(-NC0 // KCH)                    # gathers per core per stream
    NG1 = -(-NC1 // KCH)
    NG0 = -(-NG0 // GPK) * GPK              # pad to pack multiple
    NG1 = -(-NG1 // GPK) * GPK

    # edge -> (window, half) group; position within group
    ew = win_of[dst].astype(np.int64)
    group = ew * 2 + half
    eorder = np.argsort(group, kind="stable")
    g_sorted = group[eorder]
    grp_start = np.searchsorted(g_sorted, np.arange(n_win_tot * 2), side="left")
    pos_in_grp = np.arange(n_e, dtype=np.int64) - grp_start[g_sorted]

    src_s = src[eorder]
    dst_s = dst[eorder]
    half_s = half[eorder]
    w_s = ew[eorder]
    core_s = w_s // n_win
    wloc_s = w_s % n_win

    chunk_in_win = pos_in_grp // P
    slot_in_chunk = pos_in_grp % P
    T_arr = np.where(half_s == 0, T0, T1)
    assert (chunk_in_win < T_arr).all()
    chunk_core = wloc_s * T_arr + chunk_in_win
    gpos = chunk_core * P + slot_in_chunk

    idx0 = np.zeros((N_CORES, NG0 * KCH * P), np.int16)
    idx1 = np.zeros((N_CORES, NG1 * KCH * P), np.int16)

    dstloc_s = slot_of[dst_s].astype(np.int64)

    m0 = half_s == 0
    m1 = ~m0
    idx0[core_s[m0], gpos[m0]] = src_s[m0].astype(np.int16)
    idx1[core_s[m1], gpos[m1]] = (src_s[m1] - H0).astype(np.int16)

    # binary one-hot S (fp8): S[core, w, t, e, dslot] = 1.0
    f8 = ml_dtypes.float8_e4m3
    T = T0 + T1
    S_all = np.zeros((N_CORES, n_win, T, P, P), f8)
    # stream0 chunk t in [0,T0), stream1 chunk t in [T0,T)
    t_arr = np.where(m0, chunk_in_win, T0 + chunk_in_win)
    S_all[core_s, wloc_s, t_arr, slot_in_chunk, dstloc_s] = 1.0

    # per-core tail tables [128, n_win]: rs_d scale and wsum' bias coefficient
    rst = np.ones((N_CORES, P, n_win), np.float32)
    wst = np.zeros((N_CORES, P, n_win), np.float32)
    nodes_by_win_order = order  # nodes sorted by window
    wg_arr = win_of[nodes_by_win_order]
    slots_arr = slot_of[nodes_by_win_order]
    cores_arr = wg_arr // n_win
    wl_arr = wg_arr % n_win
    rst[cores_arr, slots_arr, wl_arr] = rs[nodes_by_win_order]
    wst[cores_arr, slots_arr, wl_arr] = wsum_full[nodes_by_win_order]

    bf16 = ml_dtypes.bfloat16
    hs = h * rs[:, None]                    # h' = diag(rs) h, exact in f32
    h0 = np.ascontiguousarray(hs[:H0].astype(bf16))
    h1 = np.ascontiguousarray(hs[H0:].astype(bf16))
    Wt = np.ascontiguousarray(W.T)
    bb = np.ascontiguousarray(np.tile(b.reshape(1, D), (P, 1)).astype(np.float32))

    def pack_idx(idx_c, NG):
        # [NG, KCH*P] -> packs of GPK wrapped gathers [NG//GPK, 128, GPK*KCH*8]
        wraps = [_wrap_idx(idx_c.reshape(NG, KCH * P)[g]) for g in range(NG)]
        packs = []
        for k in range(NG // GPK):
            packs.append(np.concatenate(wraps[k * GPK:(k + 1) * GPK], axis=1))
        return np.ascontiguousarray(np.stack(packs))

    in_maps = []
    for c in range(N_CORES):
        in_maps.append({
            "h0": h0, "h1": h1,
            "idx0": pack_idx(idx0[c], NG0),
            "idx1": pack_idx(idx1[c], NG1),
            "S": np.ascontiguousarray(
                S_all[c].transpose(0, 2, 1, 3).reshape(n_win, P, T * P)),
            "rst": np.ascontiguousarray(rst[c]),
            "wst": np.ascontiguousarray(wst[c]),
            "bb": bb,
            "Wt": Wt,
        })

    out_perm_nodes = np.full((N_CORES, n_win * P), -1, np.int64)
    out_perm_nodes[cores_arr, wl_arr * P + slots_arr] = nodes_by_win_order

    geom = dict(T0=T0, T1=T1, n_win=n_win, NG0=NG0, NG1=NG1)
    return in_maps, out_perm_nodes, geom


def _build_nc(geom):
    import concourse.bacc as bacc
    import concourse.mybir as mybir
    import concourse.tile as tile

    T0, T1 = geom["T0"], geom["T1"]
    n_win = geom["n_win"]
    NG0, NG1 = geom["NG0"], geom["NG1"]
    f32, i16 = mybir.dt.float32, mybir.dt.int16
    bf16 = mybir.dt.bfloat16
    f8 = mybir.dt.float8e4
    mul = mybir.AluOpType.mult
    add = mybir.AluOpType.add

    nc = bacc.Bacc("TRN2", target_bir_lowering=False, debug=False,
                   num_devices=N_CORES, num_swdge_queues=4,
                   dynamic_dma_scratch_size=131072)
    T = T0 + T1
    IW = KCH * 8                            # idx cols per gather (wrapped)
    h0_d = nc.declare_dram_parameter("h0", [H0, D], bf16, isOutput=False)
    h1_d = nc.declare_dram_parameter("h1", [N_NODES - H0, D], bf16, isOutput=False)
    idx0_d = nc.declare_dram_parameter("idx0", [NG0 // GPK, 128, GPK * IW], i16,
                                       isOutput=False)
    idx1_d = nc.declare_dram_parameter("idx1", [NG1 // GPK, 128, GPK * IW], i16,
                                       isOutput=False)
    S_d = nc.declare_dram_parameter("S", [n_win, P, T * P], f8, isOutput=False)
    rst_d = nc.declare_dram_parameter("rst", [P, n_win], f32, isOutput=False)
    wst_d = nc.declare_dram_parameter("wst", [P, n_win], f32, isOutput=False)
    bb_d = nc.declare_dram_parameter("bb", [P, D], f32, isOutput=False)
    Wt_d = nc.declare_dram_parameter("Wt", [D, D], f32, isOutput=False)
    out_d = nc.declare_dram_parameter("out", [n_win // 2 * P, 2 * D], bf16,
                                      isOutput=True)

    with tile.TileContext(nc) as tc:
        with (
            tc.tile_pool(name="const", bufs=1) as cpool,
            tc.tile_pool(name="xp0", bufs=3 * GPK) as xp0,
            tc.tile_pool(name="xp1", bufs=3 * GPK) as xp1,
            tc.tile_pool(name="ip", bufs=6) as ip,
            tc.tile_pool(name="sp", bufs=4) as sp,
            tc.tile_pool(name="wp", bufs=3) as wp,
            tc.tile_pool(name="ps", bufs=2, space="PSUM") as psA,
            tc.tile_pool(name="psO", bufs=2, space="PSUM") as psO,
        ):
            Wt_t = cpool.tile([D, D], f32)
            nc.sync.dma_start(out=Wt_t[:], in_=Wt_d[:])
            bb_t = cpool.tile([P, D], f32)
            nc.sync.dma_start(out=bb_t[:], in_=bb_d[:])
            rst_t = cpool.tile([P, n_win], f32)
            nc.sync.dma_start(out=rst_t[:], in_=rst_d[:])
            wst_t = cpool.tile([P, n_win], f32)
            nc.sync.dma_start(out=wst_t[:], in_=wst_d[:])

            x0_tiles = [None] * NG0
            x1_tiles = [None] * NG1
            np0_done = 0
            np1_done = 0
            qn = 0

            def issue_pack(which):
                nonlocal np0_done, np1_done, qn
                if which == 0:
                    k, idx_d, xp, tiles, tag = np0_done, idx0_d, xp0, x0_tiles, "0"
                    h_d = h0_d
                else:
                    k, idx_d, xp, tiles, tag = np1_done, idx1_d, xp1, x1_tiles, "1"
                    h_d = h1_d
                it = ip.tile([128, GPK * IW], i16, tag="i" + tag)
                nc.sync.dma_start(out=it[:], in_=idx_d[k])
                for i in range(GPK):
                    x = xp.tile([P, KCH * P], bf16, tag="x" + tag)
                    nc.gpsimd.dma_gather(
                        out_ap=x[:].rearrange("p (c e) -> p c e", e=P),
                        in_ap=h_d[:], idxs_ap=it[:, i * IW:(i + 1) * IW],
                        num_idxs=KCH * P, num_idxs_reg=KCH * P, elem_size=P,
                        queue_num=qn % 4)
                    qn += 1
                    tiles[k * GPK + i] = x
                if which == 0:
                    np0_done += 1
                else:
                    np1_done += 1

            CPP = KCH * GPK                 # chunks covered per pack
            for w in range(n_win):
                # prefetch gathers ~8 windows ahead of current consumption
                while np0_done * CPP < min((w + 9) * T0 + CPP, NG0 * KCH) \
                        and np0_done < NG0 // GPK:
                    issue_pack(0)
                while np1_done * CPP < min((w + 9) * T1 + CPP, NG1 * KCH) \
                        and np1_done < NG1 // GPK:
                    issue_pack(1)

                s_tile = sp.tile([P, T * P], f8, tag="S")
                nc.sync.dma_start(out=s_tile[:], in_=S_d[w])
                s_win = s_tile[:]

                pacc = psA.tile([P, P], f32, tag="pacc")
                mi = 0
                for t in range(T0):
                    c = w * T0 + t
                    xt = x0_tiles[c // KCH][:, (c % KCH) * P:(c % KCH + 1) * P]
                    nc.tensor.matmul(out=pacc[:], lhsT=xt,
                                     rhs=s_win[:, mi * P:(mi + 1) * P],
                                     start=mi == 0, stop=mi == T - 1)
                    mi += 1
                for t in range(T1):
                    c = w * T1 + t
                    xt = x1_tiles[c // KCH][:, (c % KCH) * P:(c % KCH + 1) * P]
                    nc.tensor.matmul(out=pacc[:], lhsT=xt,
                                     rhs=s_win[:, mi * P:(mi + 1) * P],
                                     start=mi == 0, stop=mi == T - 1)
                    mi += 1

                # tail: P^T [128 feat, 128 dst] in PSUM
                pt_sb = wp.tile([P, P], f32, tag="pt")
                nc.scalar.copy(out=pt_sb[:], in_=pacc[:])
                out_ps = psO.tile([P, P], f32, tag="ops")
                nc.tensor.matmul(out=out_ps[:], lhsT=pt_sb[:],
                                 rhs=Wt_t[:], start=True, stop=True)
                # tmp = psum + wsum'_d * b   (rank-1 bias on the DVE)
                tmp_sb = wp.tile([P, P], f32, tag="tmp")
                nc.vector.scalar_tensor_tensor(
                    out=tmp_sb[:], in0=bb_t[:],
                    scalar=wst_t[:, w:w + 1], in1=out_ps[:],
                    op0=mul, op1=add)
                # out = rs_d * tmp  (per-partition scale on the ACT copy);
                # windows are paired into one [128, 256] store (512B descs)
                if w % 2 == 0:
                    out_sb = wp.tile([P, 2 * P], bf16, tag="osb")
                nc.scalar.activation(out=out_sb[:, (w % 2) * P:(w % 2 + 1) * P],
                                     in_=tmp_sb[:],
                                     func=mybir.ActivationFunctionType.Copy,
                                     scale=rst_t[:, w:w + 1])
                if w % 2 == 1:
                    nc.sync.dma_start(
                        out=out_d[(w // 2) * P:(w // 2 + 1) * P, :],
                        in_=out_sb[:])

    nc.finalize()
    # The tile scheduler reorders instructions; DMASW sem lanes are assigned
    # round-robin in FINAL order (mod 8) and each lane's sems must stay on
    # one SWDGE queue.  Rewrite queue_num to match the final order (mod 4).
    cnt = 0
    for bb in nc.m.functions[0].blocks:
        for inst in bb.instructions:
            if type(inst).__name__ == "InstDMAGatherAnt":
                inst.queue_num = cnt % 4
                cnt += 1
    return nc


def _unpack_out(arr, geom):
    """[n_win//2*P, 2*D] pair-interleaved device output -> [n_win*P, D] f32."""
    n_win = geom["n_win"]
    return (np.asarray(arr).astype(np.float32)
            .reshape(n_win // 2, P, 2, D)
            .transpose(0, 2, 1, 3)
            .reshape(n_win * P, D))


def _get_nc(geom):
    key = tuple(sorted(geom.items()))
    if key not in _COMPILED:
        _COMPILED[key] = _build_nc(geom)
    return _COMPILED[key]


def kernel(h, W, b, edges):
    from concourse.bass_utils import run_bass_kernel_spmd

    h = np.asarray(h, dtype=np.float32)
    W = np.asarray(W, dtype=np.float32)
    b = np.asarray(b, dtype=np.float32)
    edges = np.asarray(edges)

    in_maps, out_perm_nodes, geom = _preprocess(h, W, b, edges)
    nc = _get_nc(geom)
    res = None
    last_exc = None
    for _attempt in range(3):
        try:
            res = run_bass_kernel_spmd(nc, in_maps, list(range(N_CORES)))
            break
        except Exception as e:  # transient axon/NRT hiccups
            last_exc = e
            import time
            time.sleep(2.0)
    if res is None:
        raise last_exc

    out = np.zeros((N_NODES, D), np.float32)
    for c in range(N_CORES):
        rows = out_perm_nodes[c]
        valid = rows >= 0
        core_out = _unpack_out(res.results[c]["out"], geom)
        out[rows[valid]] = core_out[valid]
    return out


# revision 28
# speedup vs baseline: 1.0158x; 1.0158x over previous
"""GCN layer (symmetric-normalized, self-loops) on 8 Trainium2 NeuronCores.

out[d] = sum_{e:(s,d)} rsqrt(deg_s*deg_d) * (h_s @ W.T + b)

Factorization (linearity of the edge aggregation), rs = deg**-0.5:
  out[d] = rs_d * [ (sum_e rs_s h_s) @ W.T + (sum_e rs_s) * b ]

Device strategy (dst-sharded, SPMD over 8 cores, one instruction stream):
  - h is pre-scaled by rs_src per row (h' = diag(rs) h, folded into the bf16
    cast) so the edge one-hot S is BINARY and can be fp8 (1 byte, exact 1.0).
  - nodes are bin-packed into fixed windows of <=128 dst nodes.
  - per chunk of 128 edges: dma_gather 128 rows of h' into SBUF partitions
    (indices are signed int16, so h is split into h0=h[:32768] / h1=rest;
    2048-idx gathers amortize the SWDGE fixed overhead; 4 gathers share one
    packed index DMA), and accumulate P^T += X'^T @ S on the PE
    (lhsT = gathered X' chunk bf16, rhs = S one-hot fp8 from DRAM).
  - per window: P^T [feat, 128] PSUM -> SBUF (ACT); out_ps = P^T.T @ W.T on
    the PE; tmp = psum + wsum'_d*b on the DVE (rank-1 bias, reads PSUM);
    out = rs_d * tmp on the ACT copy (per-partition scale), DMA out as bf16.
Host (numpy) does only index/graph-metadata preparation + the diagonal
rs pre-scale folded into the input cast.
"""

import sys

sys.path.insert(0, "/opt/trn_rl_repo")

import heapq

import numpy as np

N_NODES = 50000
D = 128
N_CORES = 8
H0 = 32768          # rows in first gather table (int16 index limit)
P = 128
KCH = 8             # chunks per dma_gather (1024 indices = SWDGE max)
GPK = 4             # gathers per packed index DMA

_COMPILED = {}


def _pack_windows(c0, c1, n_bins, cap0, cap1):
    """Assign each node to a window (bin) s.t. per-bin sums of c0/c1 stay
    under cap0/cap1 and <=128 nodes per bin.  Worst-fit greedy on the max
    utilization of the two capacities, largest nodes first."""
    order = np.argsort(-(c0 + c1), kind="stable")
    heap = [(0.0, b) for b in range(n_bins)]
    heapq.heapify(heap)
    used0 = np.zeros(n_bins, np.int64)
    used1 = np.zeros(n_bins, np.int64)
    ncnt = np.zeros(n_bins, np.int64)
    win_of = np.full(N_NODES, -1, np.int32)
    for node in order:
        a = int(c0[node])
        b = int(c1[node])
        popped = []
        placed = False
        while heap:
            _, bidx = heapq.heappop(heap)
            if used0[bidx] + a <= cap0 and used1[bidx] + b <= cap1 and ncnt[bidx] < P:
                used0[bidx] += a
                used1[bidx] += b
                ncnt[bidx] += 1
                win_of[node] = bidx
                key = max(used0[bidx] / cap0, used1[bidx] / cap1)
                heapq.heappush(heap, (key, bidx))
                placed = True
                break
            if ncnt[bidx] < P:
                popped.append((max(used0[bidx] / cap0, used1[bidx] / cap1), bidx))
            # bins at node capacity are dropped permanently
        for it in popped:
            heapq.heappush(heap, it)
        if not placed:
            return None
    return win_of


def _wrap_idx(idx_flat):
    """dma_gather index layout: idx i -> partition i%16, col i//16,
    replicated 8x across the 128 partitions (one copy per gpsimd core)."""
    w = idx_flat.reshape(-1, 16).T
    return np.ascontiguousarray(np.tile(w, (8, 1)))


def _preprocess(h, W, b, edges):
    import ml_dtypes

    src = np.concatenate([edges[0], np.arange(N_NODES, dtype=np.int64)]).astype(np.int64)
    dst = np.concatenate([edges[1], np.arange(N_NODES, dtype=np.int64)]).astype(np.int64)
    n_e = src.shape[0]

    deg = np.bincount(dst, minlength=N_NODES).astype(np.float32)
    rs = (deg ** -0.5).astype(np.float32)
    # wsum'[d] = sum_{e into d} rs_src  (bias coefficient; rs_d on ACT copy)
    wsum_full = np.bincount(dst, weights=rs[src].astype(np.float64),
                            minlength=N_NODES).astype(np.float32)

    half = (src >= H0).astype(np.int64)
    c0 = np.bincount(dst[half == 0], minlength=N_NODES)
    c1 = np.bincount(dst[half == 1], minlength=N_NODES)

    n_win = 52                              # windows per core
    n_win_tot = n_win * N_CORES
    for T0, T1 in ((11, 6), (11, 7), (12, 7), (13, 8)):
        win_of = _pack_windows(c0, c1, n_win_tot, T0 * P, T1 * P)
        if win_of is not None:
            break
    assert win_of is not None, "window packing failed"

    slot_of = np.zeros(N_NODES, np.int32)
    win_nodes_count = np.zeros(n_win_tot, np.int32)
    order = np.argsort(win_of, kind="stable")
    for node in order:
        wg = win_of[node]
        slot_of[node] = win_nodes_count[wg]
        win_nodes_count[wg] += 1

    NC0, NC1 = n_win * T0, n_win * T1       # chunks per core per stream
    NG0 = -(-NC0 // KCH)                    # gathers per core per stream
    NG1 = -(-NC1 // KCH)
    NG0 = -(-NG0 // GPK) * GPK              # pad to pack multiple
    NG1 = -(-NG1 // GPK) * GPK

    # edge -> (window, half) group; position within group
    ew = win_of[dst].astype(np.int64)
    group = ew * 2 + half
    eorder = np.argsort(group, kind="stable")
    g_sorted = group[eorder]
    grp_start = np.searchsorted(g_sorted, np.arange(n_win_tot * 2), side="left")
    pos_in_grp = np.arange(n_e, dtype=np.int64) - grp_start[g_sorted]

    src_s = src[eorder]
    dst_s = dst[eorder]
    half_s = half[eorder]
    w_s = ew[eorder]
    core_s = w_s // n_win
    wloc_s = w_s % n_win

    chunk_in_win = pos_in_grp // P
    slot_in_chunk = pos_in_grp % P
    T_arr = np.where(half_s == 0, T0, T1)
    assert (chunk_in_win < T_arr).all()
    chunk_core = wloc_s * T_arr + chunk_in_win
    gpos = chunk_core * P + slot_in_chunk

    idx0 = np.zeros((N_CORES, NG0 * KCH * P), np.int16)
    idx1 = np.zeros((N_CORES, NG1 * KCH * P), np.int16)

    dstloc_s = slot_of[dst_s].astype(np.int64)

    m0 = half_s == 0
    m1 = ~m0
    idx0[core_s[m0], gpos[m0]] = src_s[m0].astype(np.int16)
    idx1[core_s[m1], gpos[m1]] = (src_s[m1] - H0).astype(np.int16)

    # binary one-hot S (fp8): S[core, w, t, e, dslot] = 1.0
    f8 = ml_dtypes.float8_e4m3
    T = T0 + T1
    S_all = np.zeros((N_CORES, n_win, T, P, P), f8)
    # stream0 chunk t in [0,T0), stream1 chunk t in [T0,T)
    t_arr = np.where(m0, chunk_in_win, T0 + chunk_in_win)
    S_all[core_s, wloc_s, t_arr, slot_in_chunk, dstloc_s] = 1.0

    # per-core tail tables [128, n_win]: rs_d scale and wsum' bias coefficient
    rst = np.ones((N_CORES, P, n_win), np.float32)
    wst = np.zeros((N_CORES, P, n_win), np.float32)
    nodes_by_win_order = order  # nodes sorted by window
    wg_arr = win_of[nodes_by_win_order]
    slots_arr = slot_of[nodes_by_win_order]
    cores_arr = wg_arr // n_win
    wl_arr = wg_arr % n_win
    rst[cores_arr, slots_arr, wl_arr] = rs[nodes_by_win_order]
    wst[cores_arr, slots_arr, wl_arr] = wsum_full[nodes_by_win_order]

    bf16 = ml_dtypes.bfloat16
    hs = h * rs[:, None]                    # h' = diag(rs) h, exact in f32
    h0 = np.ascontiguousarray(hs[:H0].astype(bf16))
    h1 = np.ascontiguousarray(hs[H0:].astype(bf16))
    Wt = np.ascontiguousarray(W.T)
    bb = np.ascontiguousarray(np.tile(b.reshape(1, D), (P, 1)).astype(np.float32))

    def pack_idx(idx_c, NG):
        # [NG, KCH*P] -> packs of GPK wrapped gathers [NG//GPK, 128, GPK*KCH*8]
        wraps = [_wrap_idx(idx_c.reshape(NG, KCH * P)[g]) for g in range(NG)]
        packs = []
        for k in range(NG // GPK):
            packs.append(np.concatenate(wraps[k * GPK:(k + 1) * GPK], axis=1))
        return np.ascontiguousarray(np.stack(packs))

    in_maps = []
    for c in range(N_CORES):
        in_maps.append({
            "h0": h0, "h1": h1,
            "idx0": pack_idx(idx0[c], NG0),
            "idx1": pack_idx(idx1[c], NG1),
            "S": np.ascontiguousarray(
                S_all[c].transpose(0, 2, 1, 3).reshape(n_win, P, T * P)),
            "rst": np.ascontiguousarray(rst[c]),
            "wst": np.ascontiguousarray(wst[c]),
            "bb": bb,
            "Wt": Wt,
        })

    out_perm_nodes = np.full((N_CORES, n_win * P), -1, np.int64)
    out_perm_nodes[cores_arr, wl_arr * P + slots_arr] = nodes_by_win_order

    geom = dict(T0=T0, T1=T1, n_win=n_win, NG0=NG0, NG1=NG1)
    return in_maps, out_perm_nodes, geom


def _build_nc(geom):
    import concourse.bacc as bacc
    import concourse.mybir as mybir
    import concourse.tile as tile

    T0, T1 = geom["T0"], geom["T1"]
    n_win = geom["n_win"]
    NG0, NG1 = geom["NG0"], geom["NG1"]
    f32, i16 = mybir.dt.float32, mybir.dt.int16
    bf16 = mybir.dt.bfloat16
    f8 = mybir.dt.float8e4
    mul = mybir.AluOpType.mult
    add = mybir.AluOpType.add

    nc = bacc.Bacc("TRN2", target_bir_lowering=False, debug=False,
                   num_devices=N_CORES, num_swdge_queues=4,
                   dynamic_dma_scratch_size=98304)
    T = T0 + T1
    IW = KCH * 8                            # idx cols per gather (wrapped)
    h0_d = nc.declare_dram_parameter("h0", [H0, D], bf16, isOutput=False)
    h1_d = nc.declare_dram_parameter("h1", [N_NODES - H0, D], bf16, isOutput=False)
    idx0_d = nc.declare_dram_parameter("idx0", [NG0 // GPK, 128, GPK * IW], i16,
                                       isOutput=False)
    idx1_d = nc.declare_dram_parameter("idx1", [NG1 // GPK, 128, GPK * IW], i16,
                                       isOutput=False)
    S_d = nc.declare_dram_parameter("S", [n_win, P, T * P], f8, isOutput=False)
    rst_d = nc.declare_dram_parameter("rst", [P, n_win], f32, isOutput=False)
    wst_d = nc.declare_dram_parameter("wst", [P, n_win], f32, isOutput=False)
    bb_d = nc.declare_dram_parameter("bb", [P, D], f32, isOutput=False)
    Wt_d = nc.declare_dram_parameter("Wt", [D, D], f32, isOutput=False)
    out_d = nc.declare_dram_parameter("out", [n_win // 2 * P, 2 * D], bf16,
                                      isOutput=True)

    with tile.TileContext(nc) as tc:
        with (
            tc.tile_pool(name="const", bufs=1) as cpool,
            tc.tile_pool(name="xp0", bufs=3 * GPK) as xp0,
            tc.tile_pool(name="xp1", bufs=3 * GPK) as xp1,
            tc.tile_pool(name="ip", bufs=6) as ip,
            tc.tile_pool(name="sp", bufs=6) as sp,
            tc.tile_pool(name="wp", bufs=3) as wp,
            tc.tile_pool(name="ps", bufs=2, space="PSUM") as psA,
            tc.tile_pool(name="psO", bufs=2, space="PSUM") as psO,
        ):
            Wt_t = cpool.tile([D, D], f32)
            nc.sync.dma_start(out=Wt_t[:], in_=Wt_d[:])
            bb_t = cpool.tile([P, D], f32)
            nc.sync.dma_start(out=bb_t[:], in_=bb_d[:])
            rst_t = cpool.tile([P, n_win], f32)
            nc.sync.dma_start(out=rst_t[:], in_=rst_d[:])
            wst_t = cpool.tile([P, n_win], f32)
            nc.sync.dma_start(out=wst_t[:], in_=wst_d[:])

            x0_tiles = [None] * NG0
            x1_tiles = [None] * NG1
            np0_done = 0
            np1_done = 0
            qn = 0

            def issue_pack(which):
                nonlocal np0_done, np1_done, qn
                if which == 0:
                    k, idx_d, xp, tiles, tag = np0_done, idx0_d, xp0, x0_tiles, "0"
                    h_d = h0_d
                else:
                    k, idx_d, xp, tiles, tag = np1_done, idx1_d, xp1, x1_tiles, "1"
                    h_d = h1_d
                it = ip.tile([128, GPK * IW], i16, tag="i" + tag)
                nc.sync.dma_start(out=it[:], in_=idx_d[k])
                for i in range(GPK):
                    x = xp.tile([P, KCH * P], bf16, tag="x" + tag)
                    nc.gpsimd.dma_gather(
                        out_ap=x[:].rearrange("p (c e) -> p c e", e=P),
                        in_ap=h_d[:], idxs_ap=it[:, i * IW:(i + 1) * IW],
                        num_idxs=KCH * P, num_idxs_reg=KCH * P, elem_size=P,
                        queue_num=qn % 4)
                    qn += 1
                    tiles[k * GPK + i] = x
                if which == 0:
                    np0_done += 1
                else:
                    np1_done += 1

            CPP = KCH * GPK                 # chunks covered per pack
            for w in range(n_win):
                # prefetch gathers ~8 windows ahead of current consumption
                while np0_done * CPP < min((w + 9) * T0 + CPP, NG0 * KCH) \
                        and np0_done < NG0 // GPK:
                    issue_pack(0)
                while np1_done * CPP < min((w + 9) * T1 + CPP, NG1 * KCH) \
                        and np1_done < NG1 // GPK:
                    issue_pack(1)

                s_tile = sp.tile([P, T * P], f8, tag="S")
                nc.sync.dma_start(out=s_tile[:], in_=S_d[w])
                s_win = s_tile[:]

                pacc = psA.tile([P, P], f32, tag="pacc")
                mi = 0
                for t in range(T0):
                    c = w * T0 + t
                    xt = x0_tiles[c // KCH][:, (c % KCH) * P:(c % KCH + 1) * P]
                    nc.tensor.matmul(out=pacc[:], lhsT=xt,
                                     rhs=s_win[:, mi * P:(mi + 1) * P],
                                     start=mi == 0, stop=mi == T - 1)
                    mi += 1
                for t in range(T1):
                    c = w * T1 + t
                    xt = x1_tiles[c // KCH][:, (c % KCH) * P:(c % KCH + 1) * P]
                    nc.tensor.matmul(out=pacc[:], lhsT=xt,
                                     rhs=s_win[:, mi * P:(mi + 1) * P],
                                     start=mi == 0, stop=mi == T - 1)
                    mi += 1

                # tail: P^T [128 feat, 128 dst] in PSUM
                pt_sb = wp.tile([P, P], f32, tag="pt")
                nc.scalar.copy(out=pt_sb[:], in_=pacc[:])
                out_ps = psO.tile([P, P], f32, tag="ops")
                nc.tensor.matmul(out=out_ps[:], lhsT=pt_sb[:],
                                 rhs=Wt_t[:], start=True, stop=True)
                # tmp = psum + wsum'_d * b   (rank-1 bias on the DVE)
                tmp_sb = wp.tile([P, P], f32, tag="tmp")
                nc.vector.scalar_tensor_tensor(
                    out=tmp_sb[:], in0=bb_t[:],
                    scalar=wst_t[:, w:w + 1], in1=out_ps[:],
                    op0=mul, op1=add)
                # out = rs_d * tmp  (per-partition scale on the ACT copy);
                # windows are paired into one [128, 256] store (512B descs)
                if w % 2 == 0:
                    out_sb = wp.tile([P, 2 * P], bf16, tag="osb")
                nc.scalar.activation(out=out_sb[:, (w % 2) * P:(w % 2 + 1) * P],
                                     in_=tmp_sb[:],
                                     func=mybir.ActivationFunctionType.Copy,
                                     scale=rst_t[:, w:w + 1])
                if w % 2 == 1:
                    nc.sync.dma_start(
                        out=out_d[(w // 2) * P:(w // 2 + 1) * P, :],
                        in_=out_sb[:])

    nc.finalize()
    # The tile scheduler reorders instructions; DMASW sem lanes are assigned
    # round-robin in FINAL order (mod 8) and each lane's sems must stay on
    # one SWDGE queue.  Rewrite queue_num to match the final order (mod 4).
    cnt = 0
    for bb in nc.m.functions[0].blocks:
        for inst in bb.instructions:
            if type(inst).__name__ == "InstDMAGatherAnt":
                inst.queue_num = cnt % 4
                cnt += 1
    return nc


def _unpack_out(arr, geom):
    """[n_win//2*P, 2*D] pair-interleaved device output -> [n_win*P, D] f32."""
    n_win = geom["n_win"]
    return (np.asarray(arr).astype(np.float32)
            .reshape(n_win // 2, P, 2, D)
            .transpose(0, 2, 1, 3)
            .reshape(n_win * P, D))


def _get_nc(geom):
    key = tuple(sorted(geom.items()))
    if key not in _COMPILED:
        _COMPILED[key] = _build_nc(geom)
    return _COMPILED[key]


def kernel(h, W, b, edges):
    from concourse.bass_utils import run_bass_kernel_spmd

    h = np.asarray(h, dtype=np.float32)
    W = np.asarray(W, dtype=np.float32)
    b = np.asarray(b, dtype=np.float32)
    edges = np.asarray(edges)

    in_maps, out_perm_nodes, geom = _preprocess(h, W, b, edges)
    nc = _get_nc(geom)
    res = None
    last_exc = None
    for _attempt in range(3):
        try:
            res = run_bass_kernel_spmd(nc, in_maps, list(range(N_CORES)))
            break
        except Exception as e:  # transient axon/NRT hiccups
            last_exc = e
            import time
            time.sleep(2.0)
    if res is None:
        raise last_exc

    out = np.zeros((N_NODES, D), np.float32)
    for c in range(N_CORES):
        rows = out_perm_nodes[c]
        valid = rows >= 0
        core_out = _unpack_out(res.results[c]["out"], geom)
        out[rows[valid]] = core_out[valid]
    return out


# revision 34
# speedup vs baseline: 1.1000x; 1.0829x over previous
"""GCN layer (symmetric-normalized, self-loops) on 8 Trainium2 NeuronCores.

out[d] = sum_{e:(s,d)} rsqrt(deg_s*deg_d) * (h_s @ W.T + b)

Factorization (linearity of the edge aggregation), rs = deg**-0.5:
  out[d] = rs_d * [ (sum_e rs_s h_s) @ W.T + (sum_e rs_s) * b ]

Device strategy (dst-sharded, SPMD over 8 cores, one instruction stream):
  - h is pre-scaled by rs_src per row (h' = diag(rs) h, folded into the bf16
    cast) so the edge one-hot S is BINARY and can be fp8 (1 byte, exact 1.0).
  - nodes are bin-packed into fixed windows of <=128 dst nodes.
  - per chunk of 128 edges: dma_gather 128 rows of h' into SBUF partitions
    (indices are signed int16, so h is split into h0=h[:32768] / h1=rest;
    2048-idx gathers amortize the SWDGE fixed overhead; 4 gathers share one
    packed index DMA), and accumulate P^T += X'^T @ S on the PE
    (lhsT = gathered X' chunk bf16, rhs = S one-hot fp8 from DRAM).
  - per window: P^T [feat, 128] PSUM -> SBUF (ACT); out_ps = P^T.T @ W.T on
    the PE; tmp = psum + wsum'_d*b on the DVE (rank-1 bias, reads PSUM);
    out = rs_d * tmp on the ACT copy (per-partition scale), DMA out as bf16.
Host (numpy) does only index/graph-metadata preparation + the diagonal
rs pre-scale folded into the input cast.
"""

import sys

sys.path.insert(0, "/opt/trn_rl_repo")

import heapq

import numpy as np

N_NODES = 50000
D = 128
N_CORES = 8
H0 = 32768          # rows in first gather table (int16 index limit)
P = 128
KCH = 8             # chunks per dma_gather (1024 indices = SWDGE max)
GPK = 4             # gathers per packed index DMA

_COMPILED = {}


def _pack_windows(c0, c1, n_bins, cap0, cap1):
    """Assign each node to a window (bin) s.t. per-bin sums of c0/c1 stay
    under cap0/cap1 and <=128 nodes per bin.  Worst-fit greedy on the max
    utilization of the two capacities, largest nodes first."""
    order = np.argsort(-(c0 + c1), kind="stable")
    heap = [(0.0, b) for b in range(n_bins)]
    heapq.heapify(heap)
    used0 = np.zeros(n_bins, np.int64)
    used1 = np.zeros(n_bins, np.int64)
    ncnt = np.zeros(n_bins, np.int64)
    win_of = np.full(N_NODES, -1, np.int32)
    for node in order:
        a = int(c0[node])
        b = int(c1[node])
        popped = []
        placed = False
        while heap:
            _, bidx = heapq.heappop(heap)
            if used0[bidx] + a <= cap0 and used1[bidx] + b <= cap1 and ncnt[bidx] < P:
                used0[bidx] += a
                used1[bidx] += b
                ncnt[bidx] += 1
                win_of[node] = bidx
                key = max(used0[bidx] / cap0, used1[bidx] / cap1)
                heapq.heappush(heap, (key, bidx))
                placed = True
                break
            if ncnt[bidx] < P:
                popped.append((max(used0[bidx] / cap0, used1[bidx] / cap1), bidx))
            # bins at node capacity are dropped permanently
        for it in popped:
            heapq.heappush(heap, it)
        if not placed:
            return None
    return win_of


def _wrap_idx(idx_flat):
    """dma_gather index layout: idx i -> partition i%16, col i//16,
    replicated 8x across the 128 partitions (one copy per gpsimd core)."""
    w = idx_flat.reshape(-1, 16).T
    return np.ascontiguousarray(np.tile(w, (8, 1)))


def _preprocess(h, W, b, edges):
    import ml_dtypes

    src = np.concatenate([edges[0], np.arange(N_NODES, dtype=np.int64)]).astype(np.int64)
    dst = np.concatenate([edges[1], np.arange(N_NODES, dtype=np.int64)]).astype(np.int64)
    n_e = src.shape[0]

    deg = np.bincount(dst, minlength=N_NODES).astype(np.float32)
    rs = (deg ** -0.5).astype(np.float32)
    # wsum'[d] = sum_{e into d} rs_src  (bias coefficient; rs_d on ACT copy)
    wsum_full = np.bincount(dst, weights=rs[src].astype(np.float64),
                            minlength=N_NODES).astype(np.float32)

    half = (src >= H0).astype(np.int64)
    c0 = np.bincount(dst[half == 0], minlength=N_NODES)
    c1 = np.bincount(dst[half == 1], minlength=N_NODES)

    win_of = None
    for n_win, T0, T1 in ((50, 11, 6), (52, 11, 6), (52, 11, 7), (52, 12, 7),
                          (52, 13, 8)):
        win_of = _pack_windows(c0, c1, n_win * N_CORES, T0 * P, T1 * P)
        if win_of is not None:
            break
    assert win_of is not None, "window packing failed"
    n_win_tot = n_win * N_CORES

    slot_of = np.zeros(N_NODES, np.int32)
    win_nodes_count = np.zeros(n_win_tot, np.int32)
    order = np.argsort(win_of, kind="stable")
    for node in order:
        wg = win_of[node]
        slot_of[node] = win_nodes_count[wg]
        win_nodes_count[wg] += 1

    NC0, NC1 = n_win * T0, n_win * T1       # chunks per core per stream
    NG0r = -(-NC0 // KCH)                   # real gathers per core per stream
    NG1r = -(-NC1 // KCH)
    NG0 = -(-NG0r // GPK) * GPK             # DRAM idx padded to pack multiple
    NG1 = -(-NG1r // GPK) * GPK

    # edge -> (window, half) group; position within group
    ew = win_of[dst].astype(np.int64)
    group = ew * 2 + half
    eorder = np.argsort(group, kind="stable")
    g_sorted = group[eorder]
    grp_start = np.searchsorted(g_sorted, np.arange(n_win_tot * 2), side="left")
    pos_in_grp = np.arange(n_e, dtype=np.int64) - grp_start[g_sorted]

    src_s = src[eorder]
    dst_s = dst[eorder]
    half_s = half[eorder]
    w_s = ew[eorder]
    core_s = w_s // n_win
    wloc_s = w_s % n_win

    chunk_in_win = pos_in_grp // P
    slot_in_chunk = pos_in_grp % P
    T_arr = np.where(half_s == 0, T0, T1)
    assert (chunk_in_win < T_arr).all()
    chunk_core = wloc_s * T_arr + chunk_in_win
    gpos = chunk_core * P + slot_in_chunk

    idx0 = np.zeros((N_CORES, NG0 * KCH * P), np.int16)
    idx1 = np.zeros((N_CORES, NG1 * KCH * P), np.int16)

    dstloc_s = slot_of[dst_s].astype(np.int64)

    m0 = half_s == 0
    m1 = ~m0
    idx0[core_s[m0], gpos[m0]] = src_s[m0].astype(np.int16)
    idx1[core_s[m1], gpos[m1]] = (src_s[m1] - H0).astype(np.int16)

    # binary one-hot S (fp8): S[core, w, t, e, dslot] = 1.0
    f8 = ml_dtypes.float8_e4m3
    T = T0 + T1
    S_all = np.zeros((N_CORES, n_win, T, P, P), f8)
    # stream0 chunk t in [0,T0), stream1 chunk t in [T0,T)
    t_arr = np.where(m0, chunk_in_win, T0 + chunk_in_win)
    S_all[core_s, wloc_s, t_arr, slot_in_chunk, dstloc_s] = 1.0

    # per-core tail tables [128, n_win]: rs_d scale and wsum' bias coefficient
    rst = np.ones((N_CORES, P, n_win), np.float32)
    wst = np.zeros((N_CORES, P, n_win), np.float32)
    nodes_by_win_order = order  # nodes sorted by window
    wg_arr = win_of[nodes_by_win_order]
    slots_arr = slot_of[nodes_by_win_order]
    cores_arr = wg_arr // n_win
    wl_arr = wg_arr % n_win
    rst[cores_arr, slots_arr, wl_arr] = rs[nodes_by_win_order]
    wst[cores_arr, slots_arr, wl_arr] = wsum_full[nodes_by_win_order]

    bf16 = ml_dtypes.bfloat16
    hs = h * rs[:, None]                    # h' = diag(rs) h, exact in f32
    h0 = np.ascontiguousarray(hs[:H0].astype(bf16))
    h1 = np.ascontiguousarray(hs[H0:].astype(bf16))
    Wt = np.ascontiguousarray(W.T)
    bb = np.ascontiguousarray(np.tile(b.reshape(1, D), (P, 1)).astype(np.float32))

    def pack_idx(idx_c, NG):
        # [NG, KCH*P] -> packs of GPK wrapped gathers [NG//GPK, 128, GPK*KCH*8]
        wraps = [_wrap_idx(idx_c.reshape(NG, KCH * P)[g]) for g in range(NG)]
        packs = []
        for k in range(NG // GPK):
            packs.append(np.concatenate(wraps[k * GPK:(k + 1) * GPK], axis=1))
        return np.ascontiguousarray(np.stack(packs))

    in_maps = []
    for c in range(N_CORES):
        in_maps.append({
            "h0": h0, "h1": h1,
            "idx0": pack_idx(idx0[c], NG0),
            "idx1": pack_idx(idx1[c], NG1),
            "S": np.ascontiguousarray(
                S_all[c].transpose(0, 2, 1, 3).reshape(n_win, P, T * P)),
            "rst": np.ascontiguousarray(rst[c]),
            "wst": np.ascontiguousarray(wst[c]),
            "bb": bb,
            "Wt": Wt,
        })

    out_perm_nodes = np.full((N_CORES, n_win * P), -1, np.int64)
    out_perm_nodes[cores_arr, wl_arr * P + slots_arr] = nodes_by_win_order

    geom = dict(T0=T0, T1=T1, n_win=n_win, NG0=NG0, NG1=NG1,
                NG0r=NG0r, NG1r=NG1r)
    return in_maps, out_perm_nodes, geom


def _build_nc(geom):
    import concourse.bacc as bacc
    import concourse.mybir as mybir
    import concourse.tile as tile

    T0, T1 = geom["T0"], geom["T1"]
    n_win = geom["n_win"]
    NG0, NG1 = geom["NG0"], geom["NG1"]
    NG0r, NG1r = geom["NG0r"], geom["NG1r"]
    f32, i16 = mybir.dt.float32, mybir.dt.int16
    bf16 = mybir.dt.bfloat16
    f8 = mybir.dt.float8e4
    mul = mybir.AluOpType.mult
    add = mybir.AluOpType.add

    nc = bacc.Bacc("TRN2", target_bir_lowering=False, debug=False,
                   num_devices=N_CORES, num_swdge_queues=4,
                   dynamic_dma_scratch_size=98304)
    T = T0 + T1
    IW = KCH * 8                            # idx cols per gather (wrapped)
    h0_d = nc.declare_dram_parameter("h0", [H0, D], bf16, isOutput=False)
    h1_d = nc.declare_dram_parameter("h1", [N_NODES - H0, D], bf16, isOutput=False)
    idx0_d = nc.declare_dram_parameter("idx0", [NG0 // GPK, 128, GPK * IW], i16,
                                       isOutput=False)
    idx1_d = nc.declare_dram_parameter("idx1", [NG1 // GPK, 128, GPK * IW], i16,
                                       isOutput=False)
    S_d = nc.declare_dram_parameter("S", [n_win, P, T * P], f8, isOutput=False)
    rst_d = nc.declare_dram_parameter("rst", [P, n_win], f32, isOutput=False)
    wst_d = nc.declare_dram_parameter("wst", [P, n_win], f32, isOutput=False)
    bb_d = nc.declare_dram_parameter("bb", [P, D], f32, isOutput=False)
    Wt_d = nc.declare_dram_parameter("Wt", [D, D], f32, isOutput=False)
    out_d = nc.declare_dram_parameter("out", [n_win // 2 * P, 2 * D], bf16,
                                      isOutput=True)

    with tile.TileContext(nc) as tc:
        with (
            tc.tile_pool(name="const", bufs=1) as cpool,
            tc.tile_pool(name="xp0", bufs=3 * GPK) as xp0,
            tc.tile_pool(name="xp1", bufs=3 * GPK) as xp1,
            tc.tile_pool(name="ip", bufs=6) as ip,
            tc.tile_pool(name="sp", bufs=4) as sp,
            tc.tile_pool(name="wp", bufs=3) as wp,
            tc.tile_pool(name="ps", bufs=2, space="PSUM") as psA,
            tc.tile_pool(name="psO", bufs=2, space="PSUM") as psO,
        ):
            Wt_t = cpool.tile([D, D], f32)
            nc.sync.dma_start(out=Wt_t[:], in_=Wt_d[:])
            bb_t = cpool.tile([P, D], f32)
            nc.sync.dma_start(out=bb_t[:], in_=bb_d[:])
            rst_t = cpool.tile([P, n_win], f32)
            nc.sync.dma_start(out=rst_t[:], in_=rst_d[:])
            wst_t = cpool.tile([P, n_win], f32)
            nc.sync.dma_start(out=wst_t[:], in_=wst_d[:])

            x0_tiles = [None] * NG0
            x1_tiles = [None] * NG1
            np0_done = 0
            np1_done = 0
            qn = 0

            def issue_pack(which):
                nonlocal np0_done, np1_done, qn
                if which == 0:
                    k, idx_d, xp, tiles, tag = np0_done, idx0_d, xp0, x0_tiles, "0"
                    h_d = h0_d
                else:
                    k, idx_d, xp, tiles, tag = np1_done, idx1_d, xp1, x1_tiles, "1"
                    h_d = h1_d
                ng_real = NG0r if which == 0 else NG1r
                it = ip.tile([128, GPK * IW], i16, tag="i" + tag)
                nc.sync.dma_start(out=it[:], in_=idx_d[k])
                for i in range(min(GPK, ng_real - k * GPK)):
                    x = xp.tile([P, KCH * P], bf16, tag="x" + tag)
                    nc.gpsimd.dma_gather(
                        out_ap=x[:].rearrange("p (c e) -> p c e", e=P),
                        in_ap=h_d[:], idxs_ap=it[:, i * IW:(i + 1) * IW],
                        num_idxs=KCH * P, num_idxs_reg=KCH * P, elem_size=P,
                        queue_num=qn % 4)
                    qn += 1
                    tiles[k * GPK + i] = x
                if which == 0:
                    np0_done += 1
                else:
                    np1_done += 1

            CPP = KCH * GPK                 # chunks covered per pack
            for w in range(n_win):
                # prefetch gathers ~8 windows ahead of current consumption
                while np0_done * CPP < min((w + 9) * T0 + CPP, NG0 * KCH) \
                        and np0_done < NG0 // GPK:
                    issue_pack(0)
                while np1_done * CPP < min((w + 9) * T1 + CPP, NG1 * KCH) \
                        and np1_done < NG1 // GPK:
                    issue_pack(1)

                s_tile = sp.tile([P, T * P], f8, tag="S")
                nc.sync.dma_start(out=s_tile[:], in_=S_d[w])
                s_win = s_tile[:]

                pacc = psA.tile([P, P], f32, tag="pacc")
                mi = 0
                for t in range(T0):
                    c = w * T0 + t
                    xt = x0_tiles[c // KCH][:, (c % KCH) * P:(c % KCH + 1) * P]
                    nc.tensor.matmul(out=pacc[:], lhsT=xt,
                                     rhs=s_win[:, mi * P:(mi + 1) * P],
                                     start=mi == 0, stop=mi == T - 1)
                    mi += 1
                for t in range(T1):
                    c = w * T1 + t
                    xt = x1_tiles[c // KCH][:, (c % KCH) * P:(c % KCH + 1) * P]
                    nc.tensor.matmul(out=pacc[:], lhsT=xt,
                                     rhs=s_win[:, mi * P:(mi + 1) * P],
                                     start=mi == 0, stop=mi == T - 1)
                    mi += 1

                # tail: P^T [128 feat, 128 dst] in PSUM
                pt_sb = wp.tile([P, P], f32, tag="pt")
                nc.scalar.copy(out=pt_sb[:], in_=pacc[:])
                out_ps = psO.tile([P, P], f32, tag="ops")
                nc.tensor.matmul(out=out_ps[:], lhsT=pt_sb[:],
                                 rhs=Wt_t[:], start=True, stop=True)
                # tmp = psum + wsum'_d * b   (rank-1 bias on the DVE)
                tmp_sb = wp.tile([P, P], f32, tag="tmp")
                nc.vector.scalar_tensor_tensor(
                    out=tmp_sb[:], in0=bb_t[:],
                    scalar=wst_t[:, w:w + 1], in1=out_ps[:],
                    op0=mul, op1=add)
                # out = rs_d * tmp  (per-partition scale on the ACT copy);
                # windows are paired into one [128, 256] store (512B descs)
                if w % 2 == 0:
                    out_sb = wp.tile([P, 2 * P], bf16, tag="osb")
                nc.scalar.activation(out=out_sb[:, (w % 2) * P:(w % 2 + 1) * P],
                                     in_=tmp_sb[:],
                                     func=mybir.ActivationFunctionType.Copy,
                                     scale=rst_t[:, w:w + 1])
                if w % 2 == 1:
                    nc.sync.dma_start(
                        out=out_d[(w // 2) * P:(w // 2 + 1) * P, :],
                        in_=out_sb[:])

    nc.finalize()
    # The tile scheduler reorders instructions; DMASW sem lanes are assigned
    # round-robin in FINAL order (mod 8) and each lane's sems must stay on
    # one SWDGE queue.  Rewrite queue_num to match the final order (mod 4).
    cnt = 0
    for bb in nc.m.functions[0].blocks:
        for inst in bb.instructions:
            if type(inst).__name__ == "InstDMAGatherAnt":
                inst.queue_num = cnt % 4
                cnt += 1
    return nc


def _unpack_out(arr, geom):
    """[n_win//2*P, 2*D] pair-interleaved device output -> [n_win*P, D] f32."""
    n_win = geom["n_win"]
    return (np.asarray(arr).astype(np.float32)
            .reshape(n_win // 2, P, 2, D)
            .transpose(0, 2, 1, 3)
            .reshape(n_win * P, D))


def _get_nc(geom):
    key = tuple(sorted(geom.items()))
    if key not in _COMPILED:
        _COMPILED[key] = _build_nc(geom)
    return _COMPILED[key]


def kernel(h, W, b, edges):
    from concourse.bass_utils import run_bass_kernel_spmd

    h = np.asarray(h, dtype=np.float32)
    W = np.asarray(W, dtype=np.float32)
    b = np.asarray(b, dtype=np.float32)
    edges = np.asarray(edges)

    in_maps, out_perm_nodes, geom = _preprocess(h, W, b, edges)
    nc = _get_nc(geom)
    res = None
    last_exc = None
    for _attempt in range(3):
        try:
            res = run_bass_kernel_spmd(nc, in_maps, list(range(N_CORES)))
            break
        except Exception as e:  # transient axon/NRT hiccups
            last_exc = e
            import time
            time.sleep(2.0)
    if res is None:
        raise last_exc

    out = np.zeros((N_NODES, D), np.float32)
    for c in range(N_CORES):
        rows = out_perm_nodes[c]
        valid = rows >= 0
        core_out = _unpack_out(res.results[c]["out"], geom)
        out[rows[valid]] = core_out[valid]
    return out


# revision 36
# speedup vs baseline: 1.1981x; 1.0892x over previous
"""GCN layer (symmetric-normalized, self-loops) on 8 Trainium2 NeuronCores.

out[d] = sum_{e:(s,d)} rsqrt(deg_s*deg_d) * (h_s @ W.T + b)

Factorization (linearity of the edge aggregation), rs = deg**-0.5:
  out[d] = rs_d * [ (sum_e rs_s h_s) @ W.T + (sum_e rs_s) * b ]

Device strategy (dst-sharded, SPMD over 8 cores, one instruction stream):
  - h is pre-scaled by rs_src per row (h' = diag(rs) h, folded into the bf16
    cast) so the edge one-hot S is BINARY and can be fp8 (1 byte, exact 1.0).
  - nodes are bin-packed into fixed windows of <=128 dst nodes.
  - per chunk of 128 edges: dma_gather 128 rows of h' into SBUF partitions
    (indices are signed int16, so h is split into h0=h[:32768] / h1=rest;
    2048-idx gathers amortize the SWDGE fixed overhead; 4 gathers share one
    packed index DMA), and accumulate P^T += X'^T @ S on the PE
    (lhsT = gathered X' chunk bf16, rhs = S one-hot fp8 from DRAM).
  - per window: P^T [feat, 128] PSUM -> SBUF (ACT); out_ps = P^T.T @ W.T on
    the PE; tmp = psum + wsum'_d*b on the DVE (rank-1 bias, reads PSUM);
    out = rs_d * tmp on the ACT copy (per-partition scale), DMA out as bf16.
Host (numpy) does only index/graph-metadata preparation + the diagonal
rs pre-scale folded into the input cast.
"""

import sys

sys.path.insert(0, "/opt/trn_rl_repo")

import heapq

import numpy as np

N_NODES = 50000
D = 128
N_CORES = 8
H0 = 32768          # rows in first gather table (int16 index limit)
P = 128
KCH = 8             # chunks per dma_gather (1024 indices = SWDGE max)
GPK = 4             # gathers per packed index DMA

_COMPILED = {}


def _pack_windows(c0, c1, n_bins, cap0, cap1):
    """Assign each node to a window (bin) s.t. per-bin sums of c0/c1 stay
    under cap0/cap1 and <=128 nodes per bin.  Worst-fit greedy on the max
    utilization of the two capacities, largest nodes first."""
    order = np.argsort(-(c0 + c1), kind="stable")
    heap = [(0.0, b) for b in range(n_bins)]
    heapq.heapify(heap)
    used0 = np.zeros(n_bins, np.int64)
    used1 = np.zeros(n_bins, np.int64)
    ncnt = np.zeros(n_bins, np.int64)
    win_of = np.full(N_NODES, -1, np.int32)
    for node in order:
        a = int(c0[node])
        b = int(c1[node])
        popped = []
        placed = False
        while heap:
            _, bidx = heapq.heappop(heap)
            if used0[bidx] + a <= cap0 and used1[bidx] + b <= cap1 and ncnt[bidx] < P:
                used0[bidx] += a
                used1[bidx] += b
                ncnt[bidx] += 1
                win_of[node] = bidx
                key = max(used0[bidx] / cap0, used1[bidx] / cap1)
                heapq.heappush(heap, (key, bidx))
                placed = True
                break
            if ncnt[bidx] < P:
                popped.append((max(used0[bidx] / cap0, used1[bidx] / cap1), bidx))
            # bins at node capacity are dropped permanently
        for it in popped:
            heapq.heappush(heap, it)
        if not placed:
            return None
    return win_of


def _wrap_idx(idx_flat):
    """dma_gather index layout: idx i -> partition i%16, col i//16,
    replicated 8x across the 128 partitions (one copy per gpsimd core)."""
    w = idx_flat.reshape(-1, 16).T
    return np.ascontiguousarray(np.tile(w, (8, 1)))


def _preprocess(h, W, b, edges):
    import ml_dtypes

    src = np.concatenate([edges[0], np.arange(N_NODES, dtype=np.int64)]).astype(np.int64)
    dst = np.concatenate([edges[1], np.arange(N_NODES, dtype=np.int64)]).astype(np.int64)
    n_e = src.shape[0]

    deg = np.bincount(dst, minlength=N_NODES).astype(np.float32)
    rs = (deg ** -0.5).astype(np.float32)
    # wsum'[d] = sum_{e into d} rs_src  (bias coefficient; rs_d on ACT copy)
    wsum_full = np.bincount(dst, weights=rs[src].astype(np.float64),
                            minlength=N_NODES).astype(np.float32)

    half = (src >= H0).astype(np.int64)
    c0 = np.bincount(dst[half == 0], minlength=N_NODES)
    c1 = np.bincount(dst[half == 1], minlength=N_NODES)

    win_of = None
    for n_win, T0, T1 in ((52, 11, 6), (52, 11, 7), (52, 12, 7), (52, 13, 8)):
        win_of = _pack_windows(c0, c1, n_win * N_CORES, T0 * P, T1 * P)
        if win_of is not None:
            break
    assert win_of is not None, "window packing failed"
    n_win_tot = n_win * N_CORES

    slot_of = np.zeros(N_NODES, np.int32)
    win_nodes_count = np.zeros(n_win_tot, np.int32)
    order = np.argsort(win_of, kind="stable")
    for node in order:
        wg = win_of[node]
        slot_of[node] = win_nodes_count[wg]
        win_nodes_count[wg] += 1

    NC0, NC1 = n_win * T0, n_win * T1       # chunks per core per stream
    NG0r = -(-NC0 // KCH)                   # real gathers per core per stream
    NG1r = -(-NC1 // KCH)
    NG0 = -(-NG0r // GPK) * GPK             # DRAM idx padded to pack multiple
    NG1 = -(-NG1r // GPK) * GPK

    # edge -> (window, half) group; position within group
    ew = win_of[dst].astype(np.int64)
    group = ew * 2 + half
    eorder = np.argsort(group, kind="stable")
    g_sorted = group[eorder]
    grp_start = np.searchsorted(g_sorted, np.arange(n_win_tot * 2), side="left")
    pos_in_grp = np.arange(n_e, dtype=np.int64) - grp_start[g_sorted]

    src_s = src[eorder]
    dst_s = dst[eorder]
    half_s = half[eorder]
    w_s = ew[eorder]
    core_s = w_s // n_win
    wloc_s = w_s % n_win

    chunk_in_win = pos_in_grp // P
    slot_in_chunk = pos_in_grp % P
    T_arr = np.where(half_s == 0, T0, T1)
    assert (chunk_in_win < T_arr).all()
    chunk_core = wloc_s * T_arr + chunk_in_win
    gpos = chunk_core * P + slot_in_chunk

    idx0 = np.zeros((N_CORES, NG0 * KCH * P), np.int16)
    idx1 = np.zeros((N_CORES, NG1 * KCH * P), np.int16)

    dstloc_s = slot_of[dst_s].astype(np.int64)

    m0 = half_s == 0
    m1 = ~m0
    idx0[core_s[m0], gpos[m0]] = src_s[m0].astype(np.int16)
    idx1[core_s[m1], gpos[m1]] = (src_s[m1] - H0).astype(np.int16)

    # binary one-hot S (fp8): S[core, w, t, e, dslot] = 1.0
    f8 = ml_dtypes.float8_e4m3
    T = T0 + T1
    S_all = np.zeros((N_CORES, n_win, T, P, P), f8)
    # stream0 chunk t in [0,T0), stream1 chunk t in [T0,T)
    t_arr = np.where(m0, chunk_in_win, T0 + chunk_in_win)
    S_all[core_s, wloc_s, t_arr, slot_in_chunk, dstloc_s] = 1.0

    # per-core tail tables [128, n_win]: rs_d scale and wsum' bias coefficient
    rst = np.ones((N_CORES, P, n_win), np.float32)
    wst = np.zeros((N_CORES, P, n_win), np.float32)
    nodes_by_win_order = order  # nodes sorted by window
    wg_arr = win_of[nodes_by_win_order]
    slots_arr = slot_of[nodes_by_win_order]
    cores_arr = wg_arr // n_win
    wl_arr = wg_arr % n_win
    rst[cores_arr, slots_arr, wl_arr] = rs[nodes_by_win_order]
    wst[cores_arr, slots_arr, wl_arr] = wsum_full[nodes_by_win_order]

    bf16 = ml_dtypes.bfloat16
    hs = h * rs[:, None]                    # h' = diag(rs) h, exact in f32
    h0 = np.ascontiguousarray(hs[:H0].astype(bf16))
    h1 = np.ascontiguousarray(hs[H0:].astype(bf16))
    Wt = np.ascontiguousarray(W.T)
    bb = np.ascontiguousarray(np.tile(b.reshape(1, D), (P, 1)).astype(np.float32))

    def pack_idx(idx_c, NG):
        # [NG, KCH*P] -> packs of GPK wrapped gathers [NG//GPK, 128, GPK*KCH*8]
        wraps = [_wrap_idx(idx_c.reshape(NG, KCH * P)[g]) for g in range(NG)]
        packs = []
        for k in range(NG // GPK):
            packs.append(np.concatenate(wraps[k * GPK:(k + 1) * GPK], axis=1))
        return np.ascontiguousarray(np.stack(packs))

    in_maps = []
    for c in range(N_CORES):
        in_maps.append({
            "h0": h0, "h1": h1,
            "idx0": pack_idx(idx0[c], NG0),
            "idx1": pack_idx(idx1[c], NG1),
            "S": np.ascontiguousarray(
                S_all[c].transpose(0, 2, 1, 3).reshape(n_win, P, T * P)),
            "rst": np.ascontiguousarray(rst[c]),
            "wst": np.ascontiguousarray(wst[c]),
            "bb": bb,
            "Wt": Wt,
        })

    out_perm_nodes = np.full((N_CORES, n_win * P), -1, np.int64)
    out_perm_nodes[cores_arr, wl_arr * P + slots_arr] = nodes_by_win_order

    geom = dict(T0=T0, T1=T1, n_win=n_win, NG0=NG0, NG1=NG1,
                NG0r=NG0r, NG1r=NG1r)
    return in_maps, out_perm_nodes, geom


def _build_nc(geom):
    import concourse.bacc as bacc
    import concourse.mybir as mybir
    import concourse.tile as tile

    T0, T1 = geom["T0"], geom["T1"]
    n_win = geom["n_win"]
    NG0, NG1 = geom["NG0"], geom["NG1"]
    NG0r, NG1r = geom["NG0r"], geom["NG1r"]
    f32, i16 = mybir.dt.float32, mybir.dt.int16
    bf16 = mybir.dt.bfloat16
    f8 = mybir.dt.float8e4
    mul = mybir.AluOpType.mult
    add = mybir.AluOpType.add

    nc = bacc.Bacc("TRN2", target_bir_lowering=False, debug=False,
                   num_devices=N_CORES, num_swdge_queues=4,
                   dynamic_dma_scratch_size=98304)
    T = T0 + T1
    IW = KCH * 8                            # idx cols per gather (wrapped)
    h0_d = nc.declare_dram_parameter("h0", [H0, D], bf16, isOutput=False)
    h1_d = nc.declare_dram_parameter("h1", [N_NODES - H0, D], bf16, isOutput=False)
    idx0_d = nc.declare_dram_parameter("idx0", [NG0 // GPK, 128, GPK * IW], i16,
                                       isOutput=False)
    idx1_d = nc.declare_dram_parameter("idx1", [NG1 // GPK, 128, GPK * IW], i16,
                                       isOutput=False)
    S_d = nc.declare_dram_parameter("S", [n_win, P, T * P], f8, isOutput=False)
    rst_d = nc.declare_dram_parameter("rst", [P, n_win], f32, isOutput=False)
    wst_d = nc.declare_dram_parameter("wst", [P, n_win], f32, isOutput=False)
    bb_d = nc.declare_dram_parameter("bb", [P, D], f32, isOutput=False)
    Wt_d = nc.declare_dram_parameter("Wt", [D, D], f32, isOutput=False)
    out_d = nc.declare_dram_parameter("out", [n_win // 2 * P, 2 * D], bf16,
                                      isOutput=True)

    with tile.TileContext(nc) as tc:
        with (
            tc.tile_pool(name="const", bufs=1) as cpool,
            tc.tile_pool(name="xp0", bufs=3 * GPK) as xp0,
            tc.tile_pool(name="xp1", bufs=3 * GPK) as xp1,
            tc.tile_pool(name="ip", bufs=6) as ip,
            tc.tile_pool(name="sp", bufs=4) as sp,
            tc.tile_pool(name="wp", bufs=3) as wp,
            tc.tile_pool(name="ps", bufs=2, space="PSUM") as psA,
            tc.tile_pool(name="psO", bufs=2, space="PSUM") as psO,
        ):
            Wt_t = cpool.tile([D, D], f32)
            nc.sync.dma_start(out=Wt_t[:], in_=Wt_d[:])
            bb_t = cpool.tile([P, D], f32)
            nc.sync.dma_start(out=bb_t[:], in_=bb_d[:])
            rst_t = cpool.tile([P, n_win], f32)
            nc.sync.dma_start(out=rst_t[:], in_=rst_d[:])
            wst_t = cpool.tile([P, n_win], f32)
            nc.sync.dma_start(out=wst_t[:], in_=wst_d[:])

            x0_tiles = [None] * NG0
            x1_tiles = [None] * NG1
            np0_done = 0
            np1_done = 0
            qn = 0

            def issue_pack(which):
                nonlocal np0_done, np1_done, qn
                if which == 0:
                    k, idx_d, xp, tiles, tag = np0_done, idx0_d, xp0, x0_tiles, "0"
                    h_d = h0_d
                else:
                    k, idx_d, xp, tiles, tag = np1_done, idx1_d, xp1, x1_tiles, "1"
                    h_d = h1_d
                it = ip.tile([128, GPK * IW], i16, tag="i" + tag)
                nc.sync.dma_start(out=it[:], in_=idx_d[k])
                for i in range(GPK):
                    x = xp.tile([P, KCH * P], bf16, tag="x" + tag)
                    nc.gpsimd.dma_gather(
                        out_ap=x[:].rearrange("p (c e) -> p c e", e=P),
                        in_ap=h_d[:], idxs_ap=it[:, i * IW:(i + 1) * IW],
                        num_idxs=KCH * P, num_idxs_reg=KCH * P, elem_size=P,
                        queue_num=qn % 4)
                    qn += 1
                    tiles[k * GPK + i] = x
                if which == 0:
                    np0_done += 1
                else:
                    np1_done += 1

            CPP = KCH * GPK                 # chunks covered per pack
            for w in range(n_win):
                # prefetch gathers ~8 windows ahead of current consumption
                while np0_done * CPP < min((w + 9) * T0 + CPP, NG0 * KCH) \
                        and np0_done < NG0 // GPK:
                    issue_pack(0)
                while np1_done * CPP < min((w + 9) * T1 + CPP, NG1 * KCH) \
                        and np1_done < NG1 // GPK:
                    issue_pack(1)

                s_tile = sp.tile([P, T * P], f8, tag="S")
                nc.sync.dma_start(out=s_tile[:], in_=S_d[w])
                s_win = s_tile[:]

                pacc = psA.tile([P, P], f32, tag="pacc")
                mi = 0
                for t in range(T0):
                    c = w * T0 + t
                    xt = x0_tiles[c // KCH][:, (c % KCH) * P:(c % KCH + 1) * P]
                    nc.tensor.matmul(out=pacc[:], lhsT=xt,
                                     rhs=s_win[:, mi * P:(mi + 1) * P],
                                     start=mi == 0, stop=mi == T - 1)
                    mi += 1
                for t in range(T1):
                    c = w * T1 + t
                    xt = x1_tiles[c // KCH][:, (c % KCH) * P:(c % KCH + 1) * P]
                    nc.tensor.matmul(out=pacc[:], lhsT=xt,
                                     rhs=s_win[:, mi * P:(mi + 1) * P],
                                     start=mi == 0, stop=mi == T - 1)
                    mi += 1

                # tail: P^T [128 feat, 128 dst] in PSUM
                pt_sb = wp.tile([P, P], f32, tag="pt")
                nc.scalar.copy(out=pt_sb[:], in_=pacc[:])
                out_ps = psO.tile([P, P], f32, tag="ops")
                nc.tensor.matmul(out=out_ps[:], lhsT=pt_sb[:],
                                 rhs=Wt_t[:], start=True, stop=True)
                # tmp = psum + wsum'_d * b   (rank-1 bias on the DVE)
                tmp_sb = wp.tile([P, P], f32, tag="tmp")
                nc.vector.scalar_tensor_tensor(
                    out=tmp_sb[:], in0=bb_t[:],
                    scalar=wst_t[:, w:w + 1], in1=out_ps[:],
                    op0=mul, op1=add)
                # out = rs_d * tmp  (per-partition scale on the ACT copy);
                # windows are paired into one [128, 256] store (512B descs)
                if w % 2 == 0:
                    out_sb = wp.tile([P, 2 * P], bf16, tag="osb")
                nc.scalar.activation(out=out_sb[:, (w % 2) * P:(w % 2 + 1) * P],
                                     in_=tmp_sb[:],
                                     func=mybir.ActivationFunctionType.Copy,
                                     scale=rst_t[:, w:w + 1])
                if w % 2 == 1:
                    nc.sync.dma_start(
                        out=out_d[(w // 2) * P:(w // 2 + 1) * P, :],
                        in_=out_sb[:])

    nc.finalize()
    # The tile scheduler reorders instructions; DMASW sem lanes are assigned
    # round-robin in FINAL order (mod 8) and each lane's sems must stay on
    # one SWDGE queue.  Rewrite queue_num to match the final order (mod 4).
    cnt = 0
    for bb in nc.m.functions[0].blocks:
        for inst in bb.instructions:
            if type(inst).__name__ == "InstDMAGatherAnt":
                inst.queue_num = cnt % 4
                cnt += 1
    return nc


def _unpack_out(arr, geom):
    """[n_win//2*P, 2*D] pair-interleaved device output -> [n_win*P, D] f32."""
    n_win = geom["n_win"]
    return (np.asarray(arr).astype(np.float32)
            .reshape(n_win // 2, P, 2, D)
            .transpose(0, 2, 1, 3)
            .reshape(n_win * P, D))


def _get_nc(geom):
    key = tuple(sorted(geom.items()))
    if key not in _COMPILED:
        _COMPILED[key] = _build_nc(geom)
    return _COMPILED[key]


def kernel(h, W, b, edges):
    from concourse.bass_utils import run_bass_kernel_spmd

    h = np.asarray(h, dtype=np.float32)
    W = np.asarray(W, dtype=np.float32)
    b = np.asarray(b, dtype=np.float32)
    edges = np.asarray(edges)

    in_maps, out_perm_nodes, geom = _preprocess(h, W, b, edges)
    nc = _get_nc(geom)
    res = None
    last_exc = None
    for _attempt in range(3):
        try:
            res = run_bass_kernel_spmd(nc, in_maps, list(range(N_CORES)))
            break
        except Exception as e:  # transient axon/NRT hiccups
            last_exc = e
            import time
            time.sleep(2.0)
    if res is None:
        raise last_exc

    out = np.zeros((N_NODES, D), np.float32)
    for c in range(N_CORES):
        rows = out_perm_nodes[c]
        valid = rows >= 0
        core_out = _unpack_out(res.results[c]["out"], geom)
        out[rows[valid]] = core_out[valid]
    return out


# revision 40
# speedup vs baseline: 1.2124x; 1.0120x over previous
"""GCN layer (symmetric-normalized, self-loops) on 8 Trainium2 NeuronCores.

out[d] = sum_{e:(s,d)} rsqrt(deg_s*deg_d) * (h_s @ W.T + b)

Factorization (linearity of the edge aggregation), rs = deg**-0.5:
  out[d] = rs_d * [ (sum_e rs_s h_s) @ W.T + (sum_e rs_s) * b ]

Device strategy (dst-sharded, SPMD over 8 cores, one instruction stream):
  - h is pre-scaled by rs_src per row (h' = diag(rs) h, folded into the bf16
    cast) so the edge one-hot S is BINARY and can be fp8 (1 byte, exact 1.0).
  - nodes are bin-packed into fixed windows of <=128 dst nodes.
  - per chunk of 128 edges: dma_gather 128 rows of h' into SBUF partitions
    (indices are signed int16, so h is split into h0=h[:32768] / h1=rest;
    2048-idx gathers amortize the SWDGE fixed overhead; 4 gathers share one
    packed index DMA), and accumulate P^T += X'^T @ S on the PE
    (lhsT = gathered X' chunk bf16, rhs = S one-hot fp8 from DRAM).
  - per window: P^T [feat, 128] PSUM -> SBUF (ACT); out_ps = P^T.T @ W.T on
    the PE; tmp = psum + wsum'_d*b on the DVE (rank-1 bias, reads PSUM);
    out = rs_d * tmp on the ACT copy (per-partition scale), DMA out as bf16.
Host (numpy) does only index/graph-metadata preparation + the diagonal
rs pre-scale folded into the input cast.
"""

import sys

sys.path.insert(0, "/opt/trn_rl_repo")

import heapq

import numpy as np

N_NODES = 50000
D = 128
N_CORES = 8
H0 = 32768          # rows in first gather table (int16 index limit)
P = 128
KCH = 8             # chunks per dma_gather (1024 indices = SWDGE max)
GPK = 4             # gathers per packed index DMA

_COMPILED = {}


def _pack_windows(c0, c1, n_bins, cap0, cap1):
    """Assign each node to a window (bin) s.t. per-bin sums of c0/c1 stay
    under cap0/cap1 and <=128 nodes per bin.  Worst-fit greedy on the max
    utilization of the two capacities, largest nodes first."""
    order = np.argsort(-(c0 + c1), kind="stable")
    heap = [(0.0, b) for b in range(n_bins)]
    heapq.heapify(heap)
    used0 = np.zeros(n_bins, np.int64)
    used1 = np.zeros(n_bins, np.int64)
    ncnt = np.zeros(n_bins, np.int64)
    win_of = np.full(N_NODES, -1, np.int32)
    for node in order:
        a = int(c0[node])
        b = int(c1[node])
        popped = []
        placed = False
        while heap:
            _, bidx = heapq.heappop(heap)
            if used0[bidx] + a <= cap0 and used1[bidx] + b <= cap1 and ncnt[bidx] < P:
                used0[bidx] += a
                used1[bidx] += b
                ncnt[bidx] += 1
                win_of[node] = bidx
                key = max(used0[bidx] / cap0, used1[bidx] / cap1)
                heapq.heappush(heap, (key, bidx))
                placed = True
                break
            if ncnt[bidx] < P:
                popped.append((max(used0[bidx] / cap0, used1[bidx] / cap1), bidx))
            # bins at node capacity are dropped permanently
        for it in popped:
            heapq.heappush(heap, it)
        if not placed:
            return None
    return win_of


def _wrap_idx(idx_flat):
    """dma_gather index layout: idx i -> partition i%16, col i//16,
    replicated 8x across the 128 partitions (one copy per gpsimd core)."""
    w = idx_flat.reshape(-1, 16).T
    return np.ascontiguousarray(np.tile(w, (8, 1)))


def _preprocess(h, W, b, edges):
    import ml_dtypes

    src = np.concatenate([edges[0], np.arange(N_NODES, dtype=np.int64)]).astype(np.int64)
    dst = np.concatenate([edges[1], np.arange(N_NODES, dtype=np.int64)]).astype(np.int64)
    n_e = src.shape[0]

    deg = np.bincount(dst, minlength=N_NODES).astype(np.float32)
    rs = (deg ** -0.5).astype(np.float32)
    # wsum'[d] = sum_{e into d} rs_src  (bias coefficient; rs_d on ACT copy)
    wsum_full = np.bincount(dst, weights=rs[src].astype(np.float64),
                            minlength=N_NODES).astype(np.float32)

    half = (src >= H0).astype(np.int64)
    c0 = np.bincount(dst[half == 0], minlength=N_NODES)
    c1 = np.bincount(dst[half == 1], minlength=N_NODES)

    win_of = None
    for n_win, T0, T1 in ((52, 11, 6), (52, 11, 7), (52, 12, 7), (52, 13, 8)):
        win_of = _pack_windows(c0, c1, n_win * N_CORES, T0 * P, T1 * P)
        if win_of is not None:
            break
    assert win_of is not None, "window packing failed"
    n_win_tot = n_win * N_CORES

    slot_of = np.zeros(N_NODES, np.int32)
    win_nodes_count = np.zeros(n_win_tot, np.int32)
    order = np.argsort(win_of, kind="stable")
    for node in order:
        wg = win_of[node]
        slot_of[node] = win_nodes_count[wg]
        win_nodes_count[wg] += 1

    NC0, NC1 = n_win * T0, n_win * T1       # chunks per core per stream
    NG0r = -(-NC0 // KCH)                   # real gathers per core per stream
    NG1r = -(-NC1 // KCH)
    NG0 = -(-NG0r // GPK) * GPK             # DRAM idx padded to pack multiple
    NG1 = -(-NG1r // GPK) * GPK

    # edge -> (window, half) group; position within group
    ew = win_of[dst].astype(np.int64)
    group = ew * 2 + half
    eorder = np.argsort(group, kind="stable")
    g_sorted = group[eorder]
    grp_start = np.searchsorted(g_sorted, np.arange(n_win_tot * 2), side="left")
    pos_in_grp = np.arange(n_e, dtype=np.int64) - grp_start[g_sorted]

    src_s = src[eorder]
    dst_s = dst[eorder]
    half_s = half[eorder]
    w_s = ew[eorder]
    core_s = w_s // n_win
    wloc_s = w_s % n_win

    chunk_in_win = pos_in_grp // P
    slot_in_chunk = pos_in_grp % P
    T_arr = np.where(half_s == 0, T0, T1)
    assert (chunk_in_win < T_arr).all()
    chunk_core = wloc_s * T_arr + chunk_in_win
    gpos = chunk_core * P + slot_in_chunk

    idx0 = np.zeros((N_CORES, NG0 * KCH * P), np.int16)
    idx1 = np.zeros((N_CORES, NG1 * KCH * P), np.int16)

    dstloc_s = slot_of[dst_s].astype(np.int64)

    m0 = half_s == 0
    m1 = ~m0
    idx0[core_s[m0], gpos[m0]] = src_s[m0].astype(np.int16)
    idx1[core_s[m1], gpos[m1]] = (src_s[m1] - H0).astype(np.int16)

    # binary one-hot S (fp8): S[core, w, t, e, dslot] = 1.0
    f8 = ml_dtypes.float8_e4m3
    T = T0 + T1
    S_all = np.zeros((N_CORES, n_win, T, P, P), f8)
    # stream0 chunk t in [0,T0), stream1 chunk t in [T0,T)
    t_arr = np.where(m0, chunk_in_win, T0 + chunk_in_win)
    S_all[core_s, wloc_s, t_arr, slot_in_chunk, dstloc_s] = 1.0

    # per-core tail tables [128, n_win]: rs_d scale and wsum' bias coefficient
    rst = np.ones((N_CORES, P, n_win), np.float32)
    wst = np.zeros((N_CORES, P, n_win), np.float32)
    nodes_by_win_order = order  # nodes sorted by window
    wg_arr = win_of[nodes_by_win_order]
    slots_arr = slot_of[nodes_by_win_order]
    cores_arr = wg_arr // n_win
    wl_arr = wg_arr % n_win
    rst[cores_arr, slots_arr, wl_arr] = rs[nodes_by_win_order]
    wst[cores_arr, slots_arr, wl_arr] = wsum_full[nodes_by_win_order]

    bf16 = ml_dtypes.bfloat16
    hs = h * rs[:, None]                    # h' = diag(rs) h, exact in f32
    h0 = np.ascontiguousarray(hs[:H0].astype(bf16))
    h1 = np.ascontiguousarray(hs[H0:].astype(bf16))
    Wt = np.ascontiguousarray(W.T)
    bb = np.ascontiguousarray(np.tile(b.reshape(1, D), (P, 1)).astype(np.float32))

    def pack_idx(idx_c, NG):
        # [NG, KCH*P] -> packs of GPK wrapped gathers [NG//GPK, 128, GPK*KCH*8]
        wraps = [_wrap_idx(idx_c.reshape(NG, KCH * P)[g]) for g in range(NG)]
        packs = []
        for k in range(NG // GPK):
            packs.append(np.concatenate(wraps[k * GPK:(k + 1) * GPK], axis=1))
        return np.ascontiguousarray(np.stack(packs))

    in_maps = []
    for c in range(N_CORES):
        in_maps.append({
            "h0": h0, "h1": h1,
            "idx0": pack_idx(idx0[c], NG0),
            "idx1": pack_idx(idx1[c], NG1),
            "S": np.ascontiguousarray(
                S_all[c].transpose(0, 2, 1, 3).reshape(n_win, P, T * P)),
            "rst": np.ascontiguousarray(rst[c]),
            "wst": np.ascontiguousarray(wst[c]),
            "bb": bb,
            "Wt": Wt,
        })

    out_perm_nodes = np.full((N_CORES, n_win * P), -1, np.int64)
    out_perm_nodes[cores_arr, wl_arr * P + slots_arr] = nodes_by_win_order

    geom = dict(T0=T0, T1=T1, n_win=n_win, NG0=NG0, NG1=NG1,
                NG0r=NG0r, NG1r=NG1r)
    return in_maps, out_perm_nodes, geom


def _build_nc(geom):
    import concourse.bacc as bacc
    import concourse.mybir as mybir
    import concourse.tile as tile

    T0, T1 = geom["T0"], geom["T1"]
    n_win = geom["n_win"]
    NG0, NG1 = geom["NG0"], geom["NG1"]
    NG0r, NG1r = geom["NG0r"], geom["NG1r"]
    f32, i16 = mybir.dt.float32, mybir.dt.int16
    bf16 = mybir.dt.bfloat16
    f8 = mybir.dt.float8e4
    mul = mybir.AluOpType.mult
    add = mybir.AluOpType.add

    nc = bacc.Bacc("TRN2", target_bir_lowering=False, debug=False,
                   num_devices=N_CORES, num_swdge_queues=4,
                   dynamic_dma_scratch_size=98304)
    T = T0 + T1
    IW = KCH * 8                            # idx cols per gather (wrapped)
    h0_d = nc.declare_dram_parameter("h0", [H0, D], bf16, isOutput=False)
    h1_d = nc.declare_dram_parameter("h1", [N_NODES - H0, D], bf16, isOutput=False)
    idx0_d = nc.declare_dram_parameter("idx0", [NG0 // GPK, 128, GPK * IW], i16,
                                       isOutput=False)
    idx1_d = nc.declare_dram_parameter("idx1", [NG1 // GPK, 128, GPK * IW], i16,
                                       isOutput=False)
    S_d = nc.declare_dram_parameter("S", [n_win, P, T * P], f8, isOutput=False)
    rst_d = nc.declare_dram_parameter("rst", [P, n_win], f32, isOutput=False)
    wst_d = nc.declare_dram_parameter("wst", [P, n_win], f32, isOutput=False)
    bb_d = nc.declare_dram_parameter("bb", [P, D], f32, isOutput=False)
    Wt_d = nc.declare_dram_parameter("Wt", [D, D], f32, isOutput=False)
    out_d = nc.declare_dram_parameter("out", [n_win // 2 * P, 2 * D], bf16,
                                      isOutput=True)

    with tile.TileContext(nc) as tc:
        with (
            tc.tile_pool(name="const", bufs=1) as cpool,
            tc.tile_pool(name="xp0", bufs=3 * GPK) as xp0,
            tc.tile_pool(name="xp1", bufs=3 * GPK) as xp1,
            tc.tile_pool(name="ip", bufs=6) as ip,
            tc.tile_pool(name="sp", bufs=4) as sp,
            tc.tile_pool(name="wp", bufs=3) as wp,
            tc.tile_pool(name="ps", bufs=3, space="PSUM") as psA,
            tc.tile_pool(name="psO", bufs=2, space="PSUM") as psO,
        ):
            x0_tiles = [None] * NG0
            x1_tiles = [None] * NG1
            np0_done = 0
            np1_done = 0
            qn = 0

            def issue_pack(which):
                nonlocal np0_done, np1_done, qn
                if which == 0:
                    k, idx_d, xp, tiles, tag = np0_done, idx0_d, xp0, x0_tiles, "0"
                    h_d = h0_d
                else:
                    k, idx_d, xp, tiles, tag = np1_done, idx1_d, xp1, x1_tiles, "1"
                    h_d = h1_d
                it = ip.tile([128, GPK * IW], i16, tag="i" + tag)
                nc.sync.dma_start(out=it[:], in_=idx_d[k])
                for i in range(GPK):
                    x = xp.tile([P, KCH * P], bf16, tag="x" + tag)
                    nc.gpsimd.dma_gather(
                        out_ap=x[:].rearrange("p (c e) -> p c e", e=P),
                        in_ap=h_d[:], idxs_ap=it[:, i * IW:(i + 1) * IW],
                        num_idxs=KCH * P, num_idxs_reg=KCH * P, elem_size=P,
                        queue_num=qn % 4)
                    qn += 1
                    tiles[k * GPK + i] = x
                if which == 0:
                    np0_done += 1
                else:
                    np1_done += 1

            # first gather packs go ahead of the const loads so the DMA
            # pipeline ramps immediately
            issue_pack(0)
            issue_pack(0)
            issue_pack(1)

            Wt_t = cpool.tile([D, D], f32)
            nc.sync.dma_start(out=Wt_t[:], in_=Wt_d[:])
            bb_t = cpool.tile([P, D], f32)
            nc.sync.dma_start(out=bb_t[:], in_=bb_d[:])
            rst_t = cpool.tile([P, n_win], f32)
            nc.sync.dma_start(out=rst_t[:], in_=rst_d[:])
            wst_t = cpool.tile([P, n_win], f32)
            nc.sync.dma_start(out=wst_t[:], in_=wst_d[:])

            CPP = KCH * GPK                 # chunks covered per pack
            for w in range(n_win):
                # prefetch gathers ~8 windows ahead of current consumption
                while np0_done * CPP < min((w + 9) * T0 + CPP, NG0 * KCH) \
                        and np0_done < NG0 // GPK:
                    issue_pack(0)
                while np1_done * CPP < min((w + 9) * T1 + CPP, NG1 * KCH) \
                        and np1_done < NG1 // GPK:
                    issue_pack(1)

                s_tile = sp.tile([P, T * P], f8, tag="S")
                nc.sync.dma_start(out=s_tile[:], in_=S_d[w])
                s_win = s_tile[:]

                pacc = psA.tile([P, P], f32, tag="pacc")
                mi = 0
                for t in range(T0):
                    c = w * T0 + t
                    xt = x0_tiles[c // KCH][:, (c % KCH) * P:(c % KCH + 1) * P]
                    nc.tensor.matmul(out=pacc[:], lhsT=xt,
                                     rhs=s_win[:, mi * P:(mi + 1) * P],
                                     start=mi == 0, stop=mi == T - 1)
                    mi += 1
                for t in range(T1):
                    c = w * T1 + t
                    xt = x1_tiles[c // KCH][:, (c % KCH) * P:(c % KCH + 1) * P]
                    nc.tensor.matmul(out=pacc[:], lhsT=xt,
                                     rhs=s_win[:, mi * P:(mi + 1) * P],
                                     start=mi == 0, stop=mi == T - 1)
                    mi += 1

                # tail: P^T [128 feat, 128 dst] in PSUM
                pt_sb = wp.tile([P, P], f32, tag="pt")
                nc.scalar.copy(out=pt_sb[:], in_=pacc[:])
                out_ps = psO.tile([P, P], f32, tag="ops")
                nc.tensor.matmul(out=out_ps[:], lhsT=pt_sb[:],
                                 rhs=Wt_t[:], start=True, stop=True)
                # tmp = psum + wsum'_d * b   (rank-1 bias on the DVE)
                tmp_sb = wp.tile([P, P], f32, tag="tmp")
                nc.vector.scalar_tensor_tensor(
                    out=tmp_sb[:], in0=bb_t[:],
                    scalar=wst_t[:, w:w + 1], in1=out_ps[:],
                    op0=mul, op1=add)
                # out = rs_d * tmp  (per-partition scale on the ACT copy);
                # windows are paired into one [128, 256] store (512B descs)
                if w % 2 == 0:
                    out_sb = wp.tile([P, 2 * P], bf16, tag="osb")
                nc.scalar.activation(out=out_sb[:, (w % 2) * P:(w % 2 + 1) * P],
                                     in_=tmp_sb[:],
                                     func=mybir.ActivationFunctionType.Copy,
                                     scale=rst_t[:, w:w + 1])
                if w % 2 == 1:
                    nc.scalar.dma_start(
                        out=out_d[(w // 2) * P:(w // 2 + 1) * P, :],
                        in_=out_sb[:])

    nc.finalize()
    # The tile scheduler reorders instructions; DMASW sem lanes are assigned
    # round-robin in FINAL order (mod 8) and each lane's sems must stay on
    # one SWDGE queue.  Rewrite queue_num to match the final order (mod 4).
    cnt = 0
    for bb in nc.m.functions[0].blocks:
        for inst in bb.instructions:
            if type(inst).__name__ == "InstDMAGatherAnt":
                inst.queue_num = cnt % 4
                cnt += 1
    return nc


def _unpack_out(arr, geom):
    """[n_win//2*P, 2*D] pair-interleaved device output -> [n_win*P, D] f32."""
    n_win = geom["n_win"]
    return (np.asarray(arr).astype(np.float32)
            .reshape(n_win // 2, P, 2, D)
            .transpose(0, 2, 1, 3)
            .reshape(n_win * P, D))


def _get_nc(geom):
    key = tuple(sorted(geom.items()))
    if key not in _COMPILED:
        _COMPILED[key] = _build_nc(geom)
    return _COMPILED[key]


def kernel(h, W, b, edges):
    from concourse.bass_utils import run_bass_kernel_spmd

    h = np.asarray(h, dtype=np.float32)
    W = np.asarray(W, dtype=np.float32)
    b = np.asarray(b, dtype=np.float32)
    edges = np.asarray(edges)

    in_maps, out_perm_nodes, geom = _preprocess(h, W, b, edges)
    nc = _get_nc(geom)
    res = None
    last_exc = None
    for _attempt in range(3):
        try:
            res = run_bass_kernel_spmd(nc, in_maps, list(range(N_CORES)))
            break
        except Exception as e:  # transient axon/NRT hiccups
            last_exc = e
            import time
            time.sleep(2.0)
    if res is None:
        raise last_exc

    out = np.zeros((N_NODES, D), np.float32)
    for c in range(N_CORES):
        rows = out_perm_nodes[c]
        valid = rows >= 0
        core_out = _unpack_out(res.results[c]["out"], geom)
        out[rows[valid]] = core_out[valid]
    return out
